# revision 7
# baseline (speedup 1.0000x reference)
"""Multi-head causal attention (QKV proj + RoPE + softmax) on 8 TRN2 NeuronCores.

Sharding: batch 4-way x head-group 2-way -> each core handles 1 batch and 8
contiguous heads (512 output channels). No collectives; host gathers slices.

Per-core algorithm (all matmul compute in bf16, fp32 PSUM accumulation):
  - host passes x.T (q/k/v of its batch, transposed to [emb, seq]) and W.T
    shards so every matmul contracts over the partition dim without on-device
    transposes.
  - q/k weights are row-permuted per head into [even dims | odd dims] so RoPE
    becomes: rot = x*cs + swap32(x)*sn, where swap32 is an SBUF partition-block
    swap done by DMA. The per-head dim permutation cancels in q.k dot products.
  - q/k biases are per-partition columns folded into the PSUM eviction
    (tensor_scalar add); the v bias is applied on host: P@(V+b) = P@V + l*b.
  - scores are computed transposed, S_T[k, q] = kh_T.T @ qh_T (K=64
    contraction; the A/B heads of a 128-row tile are emitted adjacently so
    they run concurrently on PE row groups).
  - softmax: exp on ScalarE from PSUM at [128,1024] granularity (no max
    subtraction: |scores| <= ~5 by construction), causal mask multiplies on
    DVE for the diagonal tiles only; fully-masked k-tiles are skipped.
  - attnT[d, q] = sum_kt V_tile[k,d|1].T @ P_T[k, q] -- a ones-column appended
    to V makes row 64 the softmax denominator for free.
  - unnormalized attnT and the denominator row go to HBM via one SBUF staging
    copy; division + final transpose + v-bias happen on host.

Scheduling (the perf rewrite vs the 313us baseline): instead of projections
-> attention phases, everything is one software pipeline. Attention units
start as soon as the m=0 projections land (~19us), and the remaining
projection pieces (8 matmul-pair quanta) are woven between the per-kt
score/exp/attnV steps so the PE never idles while ScalarE paces the exp
stream. DMA issue is split across the sync/gpsimd/vector queues so no
engine serializes input loading.
"""

import sys
import types
from collections import deque

import numpy as np
import ml_dtypes

BF16 = ml_dtypes.bfloat16
SEQ, EMB, NHEADS, BATCH = 2048, 1024, 16, 4
HD, HALF = 64, 32
HPC = 8          # heads per core
DH = 512         # output dims per core
NE = EMB // 128  # 8 contraction tiles
NT = 4           # head-pair (128-row) dout tiles
NKT = SEQ // 128  # 16 key tiles
NQC = SEQ // 512  # 4 query chunks


def _install_ntff_shim():
    """The image's antenv lacks axon_hooks; synthesize it from trn_agent_boot
    so run_bass_kernel_spmd(trace=True) can profile. Harmless if unused."""
    try:
        import antenv.axon_hooks  # noqa: F401
        return
    except ImportError:
        pass
    try:
        from trn_agent_boot.trn_boot import _ntff_profile_via_ctypes
        import antenv
    except ImportError:
        return
    hook = _ntff_profile_via_ctypes("/opt/axon/libaxon_pjrt.so")
    mod = types.ModuleType("antenv.axon_hooks")
    mod.get_axon_ntff_profile_hook = lambda: hook
    mod.set_axon_ntff_profile_hook = lambda h: None
    sys.modules["antenv.axon_hooks"] = mod
    antenv.axon_hooks = mod


_built = {}


def build(causal=True):
    if causal in _built:
        return _built[causal]
    import concourse.mybir as mybir
    import concourse.tile as tile
    from concourse import bacc

    f32 = mybir.dt.float32
    bf = mybir.dt.bfloat16
    EXP = mybir.ActivationFunctionType.Exp
    MUL = mybir.AluOpType.mult
    ADD = mybir.AluOpType.add

    nc = bacc.Bacc(None, target_bir_lowering=False, debug=False)
    with tile.TileContext(nc) as tc:
        with tc.tile_pool(name="dram", bufs=1, space="DRAM") as dram:
            xq_d = dram.tile([EMB, SEQ], bf, kind="ExternalInput", name="xq", uniquify=False)
            xk_d = dram.tile([EMB, SEQ], bf, kind="ExternalInput", name="xk", uniquify=False)
            xv_d = dram.tile([EMB, SEQ], bf, kind="ExternalInput", name="xv", uniquify=False)
            wq_d = dram.tile([EMB, DH], bf, kind="ExternalInput", name="wq", uniquify=False)
            wk_d = dram.tile([EMB, DH], bf, kind="ExternalInput", name="wk", uniquify=False)
            wv_d = dram.tile([EMB, DH], bf, kind="ExternalInput", name="wv", uniquify=False)
            bqc_d = dram.tile([128, NT], f32, kind="ExternalInput", name="bqc", uniquify=False)
            bkc_d = dram.tile([128, NT], f32, kind="ExternalInput", name="bkc", uniquify=False)
            cs_d = dram.tile([128, SEQ], bf, kind="ExternalInput", name="cs2", uniquify=False)
            sn_d = dram.tile([128, SEQ], bf, kind="ExternalInput", name="sn2", uniquify=False)
            mk_d = dram.tile([128, 4096], bf, kind="ExternalInput", name="msk", uniquify=False)
            outT_d = dram.tile([DH, SEQ], f32, kind="ExternalOutput", name="outT", uniquify=False)
            l_d = dram.tile([HPC, SEQ], f32, kind="ExternalOutput", name="lsum", uniquify=False)

            with tc.tile_pool(name="const", bufs=1) as cp, \
                 tc.tile_pool(name="xq_p", bufs=16) as xqp, \
                 tc.tile_pool(name="xk_p", bufs=16) as xkp, \
                 tc.tile_pool(name="xv_p", bufs=12) as xvp, \
                 tc.tile_pool(name="rope", bufs=3) as rp, \
                 tc.tile_pool(name="ostage", bufs=3) as op, \
                 tc.tile_pool(name="pp", bufs=2, space="PSUM") as pp, \
                 tc.tile_pool(name="sp", bufs=2, space="PSUM") as sp, \
                 tc.tile_pool(name="tA", bufs=1, space="PSUM") as ptA, \
                 tc.tile_pool(name="tB", bufs=1, space="PSUM") as ptB:

                qh = cp.tile([128, NT, SEQ], bf, name="qh")
                kh = cp.tile([128, NT, SEQ], bf, name="kh")
                vsb = cp.tile([128, NKT, HPC * 65], bf, name="vsb")
                probs = cp.tile([128, 2, NKT, 512], bf, name="probs")
                w_sb = {n: cp.tile([128, NE, DH], bf, name=f"w_{n}") for n in "qkv"}
                b_sb = {n: cp.tile([128, NT], f32, name=f"b_{n}") for n in "qk"}
                cs = cp.tile([128, SEQ], bf, name="cs")
                sn = cp.tile([128, SEQ], bf, name="sn")
                msk = cp.tile([128, 4, 2, 512], bf, name="mskt")

                # ---------------- DMA issue (queued upfront, 3 queues) ----
                # x inputs in [128, 512] sc-chunks so the first matmul's
                # operands land within ~7us and pool live-set stays small.
                xt = {"q": {}, "k": {}, "v": {}}
                pools = {"q": xqp, "k": xkp, "v": xvp}
                srcs = {"q": xq_d, "k": xk_d, "v": xv_d}

                def load_x(nm, e, c, eng):
                    t = pools[nm].tile([128, 512], bf, tag="x", name=f"x{nm}{e}_{c}")
                    eng.dma_start(out=t[:, :],
                                  in_=srcs[nm][e * 128:(e + 1) * 128,
                                               c * 512:(c + 1) * 512])
                    xt[nm][(e, c)] = t

                wd = {"q": wq_d, "k": wk_d, "v": wv_d}

                def load_w(nm, e, eng):
                    eng.dma_start(out=w_sb[nm][:, e, :],
                                  in_=wd[nm][e * 128:(e + 1) * 128, :])

                # sync queue: q/k weights+inputs, small constants; later the
                # rope swaps and output stores are appended inline.
                for e in range(NE):
                    load_w("q", e, nc.sync)
                    load_x("q", e, 0, nc.sync)
                nc.sync.dma_start(out=b_sb["q"][:, :], in_=bqc_d[:, :])
                nc.sync.dma_start(out=b_sb["k"][:, :], in_=bkc_d[:, :])
                for e in range(NE):
                    load_x("q", e, 1, nc.sync)
                for e in range(NE):
                    load_w("k", e, nc.sync)
                    load_x("k", e, 0, nc.sync)
                nc.sync.dma_start(out=cs[:, :], in_=cs_d[:, :])
                nc.sync.dma_start(out=sn[:, :], in_=sn_d[:, :])
                for e in range(NE):
                    load_x("k", e, 1, nc.sync)
                nc.sync.dma_start(out=msk[:, :, :, :],
                                  in_=mk_d[:, :].rearrange("p (d h u) -> p d h u", d=4, h=2))

                # scalar queue: v weights+inputs (idle until the first exp)
                for e in range(NE):
                    load_w("v", e, nc.scalar)
                    load_x("v", e, 0, nc.scalar)
                for e in range(NE):
                    load_x("v", e, 1, nc.scalar)

                # gpsimd queue (slow SW-dynamic DMA): only the late
                # pool-recycled sc2/3 loads, which are dependency-gated on
                # first-wave consumers anyway.
                for c in (2, 3):
                    for e in range(NE):
                        load_x("q", e, c, nc.gpsimd)
                for c in (2, 3):
                    for e in range(NE):
                        load_x("k", e, c, nc.gpsimd)
                    for e in range(NE):
                        load_x("v", e, c, nc.gpsimd)

                # only the ones-columns (col 64 of each 65-block) need init;
                # the v evictions overwrite the 64 data columns of every block
                nc.vector.memset(
                    vsb[:, :, :].rearrange("p k (h u) -> p k h u", u=65)[:, :, :, 64:65],
                    1.0)

                # ---------------- projection pieces ----------------------
                # tmp tiles per (nm, m, half): filled by two sc pieces, then
                # roped into qh/kh and released.
                tmps = {}

                def proj_piece(nm, m, sc):
                    # one [128out, 512seq] quantum: 8 e-tile matmul pairs
                    h, c = sc // 2, sc % 2
                    key = (nm, m, h)
                    if key not in tmps:
                        tmps[key] = rp.tile([128, 1024], bf, tag="tmp", bufs=10,
                                            name=f"tp{nm}{m}{h}")
                    tmp = tmps[key]
                    ps = pp.tile([128, 512], f32, tag="p", name=f"pp{nm}{m}{sc}")
                    for e in range(NE):
                        nc.tensor.matmul(
                            ps[0:64, :],
                            w_sb[nm][:, e, m * 128:m * 128 + 64],
                            xt[nm][(e, sc)][:, :],
                            start=(e == 0), stop=(e == NE - 1))
                        nc.tensor.matmul(
                            ps[64:128, :],
                            w_sb[nm][:, e, m * 128 + 64:(m + 1) * 128],
                            xt[nm][(e, sc)][:, :],
                            start=(e == 0), stop=(e == NE - 1))
                    nc.vector.tensor_scalar_add(
                        tmp[:, c * 512:(c + 1) * 512], ps[:, :],
                        b_sb[nm][:, m:m + 1])

                def rope_half(nm, m, h):
                    # consumes tmp(nm, m, h) -> writes dst[:, m, h*1024:+1024]
                    dst = qh if nm == "q" else kh
                    tmp = tmps.pop((nm, m, h))
                    lo, hi = h * 1024, (h + 1) * 1024
                    tsw = rp.tile([128, 1024], bf, tag="tsw", name=f"tw{nm}{m}{h}")
                    for blk in range(4):
                        s = blk ^ 1
                        nc.sync.dma_start(out=tsw[blk * 32:(blk + 1) * 32, :],
                                          in_=tmp[s * 32:(s + 1) * 32, :])
                    m2 = rp.tile([128, 1024], bf, tag="m2", name=f"m2{nm}{m}{h}")
                    nc.vector.tensor_tensor(dst[:, m, lo:hi], tmp[:, :], cs[:, lo:hi], MUL)
                    nc.vector.tensor_tensor(m2[:, :], tsw[:, :], sn[:, lo:hi], MUL)
                    nc.vector.tensor_tensor(dst[:, m, lo:hi], dst[:, m, lo:hi], m2[:, :], ADD)

                def v_piece(sp2):
                    # two st (128-seq) tiles of the v projection
                    for u in range(2):
                        st = 2 * sp2 + u
                        o = st * 128
                        sc, col = o // 512, o % 512
                        ps = pp.tile([128, 512], f32, tag="p", name=f"ppv{st}")
                        for e in range(NE):
                            nc.tensor.matmul(
                                ps[0:64, :],
                                xt["v"][(e, sc)][:, col:col + 64],
                                w_sb["v"][:, e, :],
                                start=(e == 0), stop=(e == NE - 1))
                            nc.tensor.matmul(
                                ps[64:128, :],
                                xt["v"][(e, sc)][:, col + 64:col + 128],
                                w_sb["v"][:, e, :],
                                start=(e == 0), stop=(e == NE - 1))
                        nc.vector.tensor_copy(
                            vsb[:, st, :]
                            .rearrange("p (h u) -> p h u", u=65)[:, :, 0:64],
                            ps[:, :].rearrange("p (h d) -> p h d", d=64))

                # ---------------- filler plumbing ------------------------
                def emit(tok):
                    kind = tok[0]
                    if kind == "Q":
                        proj_piece("q", tok[1], tok[2])
                    elif kind == "K":
                        proj_piece("k", tok[1], tok[2])
                    elif kind == "V":
                        v_piece(tok[1])
                    elif kind == "RQ":
                        rope_half("q", tok[1], tok[2])
                    elif kind == "RK":
                        rope_half("k", tok[1], tok[2])
                    done.add(tok)

                done = set()
                fifo = deque()

                def drain_until(toks):
                    while any(t not in done for t in toks):
                        emit(fifo.popleft())

                def pull(n=1):
                    for _ in range(n):
                        if fifo:
                            emit(fifo.popleft())

                # ---------------- attention unit steps -------------------
                pt = {}

                def unit_kt(t, j, kt, nkt):
                    if kt == 0:
                        pt[0] = ptA.tile([65, 512], f32, tag="t0", name=f"pt0_{t}{j}")
                        pt[1] = ptB.tile([65, 512], f32, tag="t1", name=f"pt1_{t}{j}")
                    ps = sp.tile([128, 1024], f32, tag="s", name=f"ps{t}{j}_{kt}")
                    # A/B heads write the tile's two different PSUM banks
                    # from PE row groups 0/1 -> they run concurrently.
                    for half in (0, 1):
                        po = half * 64
                        nc.tensor.matmul(
                            ps[:, half * 512:(half + 1) * 512],
                            kh[po:po + 64, t, kt * 128:(kt + 1) * 128],
                            qh[po:po + 64, t, j * 512:(j + 1) * 512],
                            start=True, stop=True)
                    nc.scalar.activation(
                        probs[:, :, kt, :],
                        ps[:, :].rearrange("p (h u) -> p h u", h=2), EXP)
                    if causal and kt >= 4 * j:
                        dd = kt - 4 * j
                        nc.vector.tensor_tensor(
                            probs[:, :, kt, :], probs[:, :, kt, :],
                            msk[:, dd, :, :], MUL)
                    for half in (0, 1):
                        lh = 2 * t + half
                        nc.tensor.matmul(
                            pt[half][:, :],
                            vsb[:, kt, lh * 65:(lh + 1) * 65],
                            probs[:, half, kt, :],
                            start=(kt == 0), stop=(kt == nkt - 1))

                def unit_end(t, j):
                    for half in (0, 1):
                        lh = 2 * t + half
                        ost = op.tile([65, 512], f32, tag="ost", name=f"os{half}_{t}{j}")
                        nc.vector.tensor_copy(ost[:, :], pt[half][:, :])
                        nc.sync.dma_start(
                            out=outT_d[lh * 64:(lh + 1) * 64, j * 512:(j + 1) * 512],
                            in_=ost[0:64, :])
                        nc.sync.dma_start(
                            out=l_d[lh:lh + 1, j * 512:(j + 1) * 512],
                            in_=ost[64:65, :])

                # ---------------- the schedule ---------------------------
                # fill: m=0 h0 projections so unit (0,0) can start early
                for tok in [("Q", 0, 0), ("Q", 0, 1), ("RQ", 0, 0),
                            ("K", 0, 0), ("K", 0, 1), ("RK", 0, 0),
                            ("V", 0), ("V", 1)]:
                    emit(tok)

                # filler fifo: wave1 = rest of h0 work, wave2 = h1 work
                for m in (1, 2):
                    fifo.extend([("Q", m, 0), ("Q", m, 1), ("RQ", m, 0),
                                 ("K", m, 0), ("K", m, 1), ("RK", m, 0)])
                fifo.extend([("V", 2), ("V", 3)])
                fifo.extend([("Q", 3, 0), ("Q", 3, 1), ("RQ", 3, 0),
                             ("K", 3, 0), ("K", 3, 1), ("RK", 3, 0)])
                fifo.extend([("Q", 0, 2), ("Q", 0, 3), ("RQ", 0, 1),
                             ("K", 0, 2), ("K", 0, 3), ("RK", 0, 1),
                             ("V", 4)])
                fifo.extend([("Q", 1, 2), ("Q", 1, 3), ("RQ", 1, 1),
                             ("K", 1, 2), ("K", 1, 3), ("RK", 1, 1),
                             ("V", 5)])
                fifo.extend([("Q", 2, 2), ("Q", 2, 3), ("RQ", 2, 1),
                             ("K", 2, 2), ("K", 2, 3), ("RK", 2, 1),
                             ("V", 6)])
                fifo.extend([("Q", 3, 2), ("Q", 3, 3), ("RQ", 3, 1),
                             ("K", 3, 2), ("K", 3, 3), ("RK", 3, 1),
                             ("V", 7)])

                def unit_deps(t, j):
                    nkt = 4 * (j + 1) if causal else NKT
                    deps = [("RQ", t, j // 2)]
                    deps += [("RK", t, h) for h in range({1: 1, 2: 2}.get((nkt + 7) // 8, 2))]
                    deps += [("V", sp2) for sp2 in range((nkt + 1) // 2)]
                    return deps

                order = [(t, j) for j in range(NQC) for t in range(NT)]
                for (t, j) in order:
                    nkt = 4 * (j + 1) if causal else NKT
                    drain_until(unit_deps(t, j))
                    for kt in range(nkt):
                        unit_kt(t, j, kt, nkt)
                        pull(1)
                    unit_end(t, j)
    _built[causal] = nc
    nc.compile()
    return nc


def _prep_core_inputs(c, q, k, v, Wq, bq, Wk, bk, Wv, bv, sin, cos):
    b, hh = c // 2, c % 2
    hs = slice(hh * DH, (hh + 1) * DH)

    perm = np.empty(DH, np.int64)
    for lh in range(HPC):
        base = (hh * HPC + lh) * HD
        perm[lh * HD:lh * HD + HALF] = base + 2 * np.arange(HALF)
        perm[lh * HD + HALF:(lh + 1) * HD] = base + 2 * np.arange(HALF) + 1

    s = 0.125  # 1/sqrt(HD), folded into the q projection
    wq = np.ascontiguousarray((Wq[perm, :] * s).T).astype(BF16)
    wk = np.ascontiguousarray(Wk[perm, :].T).astype(BF16)
    wv = np.ascontiguousarray(Wv[hs, :].T).astype(BF16)

    p32 = np.arange(128) % 32
    cs2 = cos[:, p32].T.astype(BF16)
    sgn = np.where((np.arange(128) // 32) % 2 == 0, -1.0, 1.0).astype(np.float32)
    sn2 = (sin[:, p32] * sgn[None, :]).T.astype(BF16)

    kk = np.arange(128)[:, None]
    qq = np.arange(512)[None, :]
    m1 = np.stack([(128 * d + kk <= qq) for d in range(4)], axis=1)  # [128,4,512]
    msk = np.repeat(m1[:, :, None, :], 2, axis=2).reshape(128, 4096).astype(BF16)

    return {
        "xq": np.ascontiguousarray(q[b].T).astype(BF16),
        "xk": np.ascontiguousarray(k[b].T).astype(BF16),
        "xv": np.ascontiguousarray(v[b].T).astype(BF16),
        "wq": wq, "wk": wk, "wv": wv,
        "bqc": np.ascontiguousarray((bq[perm] * s).reshape(NT, 128).T, np.float32),
        "bkc": np.ascontiguousarray(bk[perm].reshape(NT, 128).T, np.float32),
        "cs2": cs2, "sn2": sn2, "msk": msk,
    }


def prep_in_maps(q, k, v, Wq, bq, Wk, bk, Wv, bv, sin, cos):
    args = [np.asarray(a, np.float32) for a in (q, k, v, Wq, bq, Wk, bk, Wv, bv, sin, cos)]
    maps = [_prep_core_inputs(c, *args) for c in range(8)]
    return maps, args[8]  # bv needed on host in assemble()


def assemble(results, bv):
    out = np.empty((BATCH, SEQ, EMB), np.float32)
    for c in range(8):
        b, hh = c // 2, c % 2
        outT = np.asarray(results[c]["outT"], np.float32)
        l = np.asarray(results[c]["lsum"], np.float32)
        a = outT.reshape(HPC, HD, SEQ) / l[:, None, :]
        out[b, :, hh * DH:(hh + 1) * DH] = a.reshape(DH, SEQ).T \
            + bv[hh * DH:(hh + 1) * DH][None, :]
    return out


def run(in_maps, causal=True, trace=False, **kw):
    _install_ntff_shim()
    from concourse.bass_utils import run_bass_kernel_spmd
    nc = build(causal)
    return run_bass_kernel_spmd(nc, in_maps, core_ids=list(range(8)), trace=trace, **kw)


def kernel(q, k, v, Wq, bq, Wk, bk, Wv, bv, sin, cos, mask):
    in_maps, bv_f = prep_in_maps(q, k, v, Wq, bq, Wk, bk, Wv, bv, sin, cos)
    r = run(in_maps, causal=bool(mask))
    return assemble(r.results, bv_f)


# revision 19
# speedup vs baseline: 1.2117x; 1.2117x over previous
"""Multi-head causal attention (QKV proj + RoPE + softmax) on 8 TRN2 NeuronCores.

Sharding: batch 4-way x head-group 2-way -> each core handles 1 batch and 8
contiguous heads (512 output channels). No collectives; host gathers slices.

Per-core algorithm (all matmul compute in bf16, fp32 PSUM accumulation):
  - host passes x.T (q/k/v of its batch, transposed to [emb, seq]) and W.T
    shards so every matmul contracts over the partition dim without on-device
    transposes.
  - q/k weights are row-permuted per head into [even dims | odd dims] so RoPE
    becomes: rot = x*cs + swap32(x)*sn, where swap32 is an SBUF partition-block
    swap done by DMA. The per-head dim permutation cancels in q.k dot products.
  - q/k biases are per-partition columns folded into the PSUM eviction
    (tensor_scalar add); the v bias is applied on host: P@(V+b) = P@V + l*b.
  - scores are computed transposed, S_T[k, q] = kh_T.T @ qh_T (K=64
    contraction; the A/B heads of a 128-row tile are emitted adjacently so
    they run concurrently on PE row groups).
  - softmax: exp on ScalarE from PSUM at [128,1024] granularity (no max
    subtraction: |scores| <= ~5 by construction), causal mask multiplies on
    DVE for the diagonal tiles only; fully-masked k-tiles are skipped.
  - attnT[d, q] = sum_kt V_tile[k,d|1].T @ P_T[k, q] -- a ones-column appended
    to V makes row 64 the softmax denominator for free.
  - unnormalized attnT and the denominator row go to HBM via one SBUF staging
    copy; division + final transpose + v-bias happen on host.

Scheduling (the perf rewrite vs the 313us baseline): instead of projections
-> attention phases, everything is one software pipeline. Attention units
start as soon as the m=0 projections land (~19us), and the remaining
projection pieces (8 matmul-pair quanta) are woven between the per-kt
score/exp/attnV steps so the PE never idles while ScalarE paces the exp
stream. DMA issue is split across the sync/gpsimd/vector queues so no
engine serializes input loading.
"""

import sys
import types
from collections import deque

import numpy as np
import ml_dtypes

BF16 = ml_dtypes.bfloat16
SEQ, EMB, NHEADS, BATCH = 2048, 1024, 16, 4
HD, HALF = 64, 32
HPC = 8          # heads per core
DH = 512         # output dims per core
NE = EMB // 128  # 8 contraction tiles
NT = 4           # head-pair (128-row) dout tiles
NKT = SEQ // 128  # 16 key tiles
NQC = SEQ // 512  # 4 query chunks


def _install_ntff_shim():
    """The image's antenv lacks axon_hooks; synthesize it from trn_agent_boot
    so run_bass_kernel_spmd(trace=True) can profile. Harmless if unused."""
    try:
        import antenv.axon_hooks  # noqa: F401
        return
    except ImportError:
        pass
    try:
        from trn_agent_boot.trn_boot import _ntff_profile_via_ctypes
        import antenv
    except ImportError:
        return
    hook = _ntff_profile_via_ctypes("/opt/axon/libaxon_pjrt.so")
    mod = types.ModuleType("antenv.axon_hooks")
    mod.get_axon_ntff_profile_hook = lambda: hook
    mod.set_axon_ntff_profile_hook = lambda h: None
    sys.modules["antenv.axon_hooks"] = mod
    antenv.axon_hooks = mod


_built = {}


def build(causal=True):
    if causal in _built:
        return _built[causal]
    import concourse.mybir as mybir
    import concourse.tile as tile
    from concourse import bacc

    f32 = mybir.dt.float32
    bf = mybir.dt.bfloat16
    EXP = mybir.ActivationFunctionType.Exp
    MUL = mybir.AluOpType.mult
    ADD = mybir.AluOpType.add

    nc = bacc.Bacc(None, target_bir_lowering=False, debug=False)
    with tile.TileContext(nc) as tc:
        with tc.tile_pool(name="dram", bufs=1, space="DRAM") as dram:
            xq_d = dram.tile([EMB, SEQ], bf, kind="ExternalInput", name="xq", uniquify=False)
            xk_d = dram.tile([EMB, SEQ], bf, kind="ExternalInput", name="xk", uniquify=False)
            xv_d = dram.tile([EMB, SEQ], bf, kind="ExternalInput", name="xv", uniquify=False)
            wq_d = dram.tile([EMB, DH], bf, kind="ExternalInput", name="wq", uniquify=False)
            wk_d = dram.tile([EMB, DH], bf, kind="ExternalInput", name="wk", uniquify=False)
            wv_d = dram.tile([EMB, DH], bf, kind="ExternalInput", name="wv", uniquify=False)
            bqc_d = dram.tile([128, NT], f32, kind="ExternalInput", name="bqc", uniquify=False)
            bkc_d = dram.tile([128, NT], f32, kind="ExternalInput", name="bkc", uniquify=False)
            cs_d = dram.tile([128, SEQ], bf, kind="ExternalInput", name="cs2", uniquify=False)
            sn_d = dram.tile([128, SEQ], bf, kind="ExternalInput", name="sn2", uniquify=False)
            mk_d = dram.tile([128, 256], bf, kind="ExternalInput", name="msk", uniquify=False)
            outT_d = dram.tile([DH, SEQ], f32, kind="ExternalOutput", name="outT", uniquify=False)
            l_d = dram.tile([HPC, SEQ], f32, kind="ExternalOutput", name="lsum", uniquify=False)

            with tc.tile_pool(name="const", bufs=1) as cp, \
                 tc.tile_pool(name="xq_p", bufs=16) as xqp, \
                 tc.tile_pool(name="xk_p", bufs=16) as xkp, \
                 tc.tile_pool(name="xv_p", bufs=16) as xvp, \
                 tc.tile_pool(name="rope", bufs=3) as rp, \
                 tc.tile_pool(name="ostage", bufs=3) as op, \
                 tc.tile_pool(name="pp", bufs=2, space="PSUM") as pp, \
                 tc.tile_pool(name="sp", bufs=2, space="PSUM") as sp, \
                 tc.tile_pool(name="tA", bufs=1, space="PSUM") as ptA, \
                 tc.tile_pool(name="tB", bufs=1, space="PSUM") as ptB:

                qh = cp.tile([128, NT, SEQ], bf, name="qh")
                kh = cp.tile([128, NT, SEQ], bf, name="kh")
                vsb = cp.tile([128, NKT, HPC * 65], bf, name="vsb")
                probs = cp.tile([128, 2, NKT, 512], bf, name="probs")
                w_sb = {n: cp.tile([128, NE, DH], bf, name=f"w_{n}") for n in "qkv"}
                b_sb = {n: cp.tile([128, NT], f32, name=f"b_{n}") for n in "qk"}
                cs = cp.tile([128, SEQ], bf, name="cs")
                sn = cp.tile([128, SEQ], bf, name="sn")
                msk = cp.tile([128, 2, 128], bf, name="mskt")

                # ---------------- DMA issue (queued upfront, 3 queues) ----
                # x inputs in [128, 512] sc-chunks so the first matmul's
                # operands land within ~7us and pool live-set stays small.
                xt = {"q": {}, "k": {}, "v": {}}
                pools = {"q": xqp, "k": xkp, "v": xvp}
                srcs = {"q": xq_d, "k": xk_d, "v": xv_d}

                def load_x(nm, e, c, eng):
                    t = pools[nm].tile([128, 512], bf, tag="x", name=f"x{nm}{e}_{c}")
                    eng.dma_start(out=t[:, :],
                                  in_=srcs[nm][e * 128:(e + 1) * 128,
                                               c * 512:(c + 1) * 512])
                    xt[nm][(e, c)] = t

                wd = {"q": wq_d, "k": wk_d, "v": wv_d}

                def load_w(nm, e, eng):
                    eng.dma_start(out=w_sb[nm][:, e, :],
                                  in_=wd[nm][e * 128:(e + 1) * 128, :])

                # sync queue: q/k weights+inputs, small constants; the
                # pool-recycled (hence dependency-gated) xq sc2/3 go last so
                # they cannot head-of-line-block anything. Output stores are
                # appended inline later.
                for e in range(NE):
                    load_w("q", e, nc.sync)
                    load_x("q", e, 0, nc.sync)
                nc.sync.dma_start(out=b_sb["q"][:, :], in_=bqc_d[:, :])
                nc.sync.dma_start(out=b_sb["k"][:, :], in_=bkc_d[:, :])
                for e in range(NE):
                    load_x("q", e, 1, nc.sync)
                nc.sync.dma_start(out=cs[:, :], in_=cs_d[:, :])
                nc.sync.dma_start(out=sn[:, :], in_=sn_d[:, :])
                nc.sync.dma_start(out=msk[:, :, :],
                                  in_=mk_d[:, :].rearrange("p (h u) -> p h u", h=2))
                for e in range(NE):
                    load_w("k", e, nc.sync)
                    load_x("k", e, 0, nc.sync)
                for e in range(NE):
                    load_x("k", e, 1, nc.sync)

                # scalar queue: v weights+inputs and the rope swaps -- the
                # scalar engine is idle until the attention phase begins.
                for e in range(NE):
                    load_w("v", e, nc.scalar)
                    load_x("v", e, 0, nc.scalar)
                for e in range(NE):
                    load_x("v", e, 1, nc.scalar)

                # gpsimd queue (slow SW-dynamic DMA): only late
                # pool-recycled loads, dependency-gated on first-wave
                # consumers anyway.
                for c in (2, 3):
                    for e in range(NE):
                        load_x("q", e, c, nc.gpsimd)
                for c in (2, 3):
                    for e in range(NE):
                        load_x("k", e, c, nc.gpsimd)
                    for e in range(NE):
                        load_x("v", e, c, nc.gpsimd)

                # only the ones-columns (col 64 of each 65-block) need init;
                # the v evictions overwrite the 64 data columns of every block
                nc.vector.memset(
                    vsb[:, :, :].rearrange("p k (h u) -> p k h u", u=65)[:, :, :, 64:65],
                    1.0)

                # ---------------- projection pieces ----------------------
                # tmp tiles per (nm, m, half): filled by two sc pieces, then
                # roped into qh/kh and released.
                tmps = {}

                def proj_piece(nm, m, sc):
                    # one [128out, 512seq] quantum: 8 e-tile matmul pairs
                    h, c = sc // 2, sc % 2
                    key = (nm, m, h)
                    if key not in tmps:
                        tmps[key] = rp.tile([128, 1024], bf, tag="tmp", bufs=10,
                                            name=f"tp{nm}{m}{h}")
                    tmp = tmps[key]
                    ps = pp.tile([128, 512], f32, tag="p", name=f"pp{nm}{m}{sc}")
                    for e in range(NE):
                        nc.tensor.matmul(
                            ps[0:64, :],
                            w_sb[nm][:, e, m * 128:m * 128 + 64],
                            xt[nm][(e, sc)][:, :],
                            start=(e == 0), stop=(e == NE - 1))
                        nc.tensor.matmul(
                            ps[64:128, :],
                            w_sb[nm][:, e, m * 128 + 64:(m + 1) * 128],
                            xt[nm][(e, sc)][:, :],
                            start=(e == 0), stop=(e == NE - 1))
                    nc.vector.tensor_scalar_add(
                        tmp[:, c * 512:(c + 1) * 512], ps[:, :],
                        b_sb[nm][:, m:m + 1])

                def rope_half(nm, m, h):
                    # consumes tmp(nm, m, h) -> writes dst[:, m, h*1024:+1024]
                    dst = qh if nm == "q" else kh
                    tmp = tmps.pop((nm, m, h))
                    lo, hi = h * 1024, (h + 1) * 1024
                    tsw = rp.tile([128, 1024], bf, tag="tsw", name=f"tw{nm}{m}{h}")
                    for blk in range(4):
                        s = blk ^ 1
                        nc.scalar.dma_start(out=tsw[blk * 32:(blk + 1) * 32, :],
                                            in_=tmp[s * 32:(s + 1) * 32, :])
                    m2 = rp.tile([128, 1024], bf, tag="m2", name=f"m2{nm}{m}{h}")
                    nc.vector.tensor_tensor(dst[:, m, lo:hi], tmp[:, :], cs[:, lo:hi], MUL)
                    nc.vector.tensor_tensor(m2[:, :], tsw[:, :], sn[:, lo:hi], MUL)
                    nc.vector.tensor_tensor(dst[:, m, lo:hi], dst[:, m, lo:hi], m2[:, :], ADD)

                def v_piece(sp2):
                    # two st (128-seq) tiles of the v projection
                    for u in range(2):
                        st = 2 * sp2 + u
                        o = st * 128
                        sc, col = o // 512, o % 512
                        ps = pp.tile([128, 512], f32, tag="p", name=f"ppv{st}")
                        for e in range(NE):
                            nc.tensor.matmul(
                                ps[0:64, :],
                                xt["v"][(e, sc)][:, col:col + 64],
                                w_sb["v"][:, e, :],
                                start=(e == 0), stop=(e == NE - 1))
                            nc.tensor.matmul(
                                ps[64:128, :],
                                xt["v"][(e, sc)][:, col + 64:col + 128],
                                w_sb["v"][:, e, :],
                                start=(e == 0), stop=(e == NE - 1))
                        nc.vector.tensor_copy(
                            vsb[:, st, :]
                            .rearrange("p (h u) -> p h u", u=65)[:, :, 0:64],
                            ps[:, :].rearrange("p (h d) -> p h d", d=64))

                # ---------------- filler plumbing ------------------------
                def emit(tok):
                    kind = tok[0]
                    if kind == "Q":
                        proj_piece("q", tok[1], tok[2])
                    elif kind == "K":
                        proj_piece("k", tok[1], tok[2])
                    elif kind == "V":
                        v_piece(tok[1])
                    elif kind == "RQ":
                        rope_half("q", tok[1], tok[2])
                    elif kind == "RK":
                        rope_half("k", tok[1], tok[2])
                    done.add(tok)

                done = set()

                # ---------------- attention unit steps -------------------
                pt = {}

                def unit_kt(t, j, kt, nkt):
                    if kt == 0:
                        pt[0] = ptA.tile([65, 512], f32, tag="t0", name=f"pt0_{t}{j}")
                        pt[1] = ptB.tile([65, 512], f32, tag="t1", name=f"pt1_{t}{j}")
                    # causal column shrink: for diagonal tiles, query columns
                    # below o are fully masked -- skip them in the scores
                    # matmul (moving N), the exp, and the attnV accumulation
                    # (nested ranges, so PSUM accumulate stays consistent).
                    dd = kt - 4 * j if causal else -1
                    o = 128 * max(dd, 0)
                    ps = sp.tile([128, 1024], f32, tag="s", name=f"ps{t}{j}_{kt}")
                    # A/B heads write the tile's two different PSUM banks
                    # from PE row groups 0/1 -> they run concurrently.
                    for half in (0, 1):
                        po = half * 64
                        nc.tensor.matmul(
                            ps[:, half * 512 + o:(half + 1) * 512],
                            kh[po:po + 64, t, kt * 128:(kt + 1) * 128],
                            qh[po:po + 64, t, j * 512 + o:(j + 1) * 512],
                            start=True, stop=True)
                    nc.scalar.activation(
                        probs[:, :, kt, o:],
                        ps[:, :].rearrange("p (h u) -> p h u", h=2)[:, :, o:], EXP)
                    if causal and 0 <= dd:
                        # triangle mask on the single partial 128-col block
                        nc.vector.tensor_tensor(
                            probs[:, :, kt, o:o + 128], probs[:, :, kt, o:o + 128],
                            msk[:, :, :], MUL)
                    for half in (0, 1):
                        lh = 2 * t + half
                        nc.tensor.matmul(
                            pt[half][:, o:],
                            vsb[:, kt, lh * 65:(lh + 1) * 65],
                            probs[:, half, kt, o:],
                            start=(kt == 0), stop=(kt == nkt - 1))

                def unit_end(t, j):
                    for half in (0, 1):
                        lh = 2 * t + half
                        ost = op.tile([65, 512], f32, tag="ost", name=f"os{half}_{t}{j}")
                        nc.vector.tensor_copy(ost[:, :], pt[half][:, :])
                        nc.sync.dma_start(
                            out=outT_d[lh * 64:(lh + 1) * 64, j * 512:(j + 1) * 512],
                            in_=ost[0:64, :])
                        nc.sync.dma_start(
                            out=l_d[lh:lh + 1, j * 512:(j + 1) * 512],
                            in_=ost[64:65, :])

                # ---------------- the schedule ---------------------------
                # Phase 1: all projections as one contiguous hot matmul
                # stream (the PE only reaches full clock in long
                # uninterrupted bursts). Rope/evictions ride on DVE behind.
                # sc-wave-major order: all consumers of the sc0/1 x-chunks
                # run before any sc2/3 piece, so the x pools recycle without
                # blocking the in-order PE queue.
                for m in range(NT):
                    emit(("Q", m, 0)); emit(("Q", m, 1)); emit(("RQ", m, 0))
                for m in range(NT):
                    emit(("K", m, 0)); emit(("K", m, 1)); emit(("RK", m, 0))
                for sp2 in range(4):
                    emit(("V", sp2))
                for m in range(NT):
                    emit(("Q", m, 2)); emit(("Q", m, 3)); emit(("RQ", m, 1))
                for m in range(NT):
                    emit(("K", m, 2)); emit(("K", m, 3)); emit(("RK", m, 1))
                for sp2 in range(4, NKT // 2):
                    emit(("V", sp2))

                # Phase 2: pure exp-paced attention stream.
                order = [(t, j) for j in range(NQC) for t in range(NT)]
                for (t, j) in order:
                    nkt = 4 * (j + 1) if causal else NKT
                    for kt in range(nkt):
                        unit_kt(t, j, kt, nkt)
                    unit_end(t, j)
    _built[causal] = nc
    nc.compile()
    return nc


def _prep_core_inputs(c, q, k, v, Wq, bq, Wk, bk, Wv, bv, sin, cos):
    b, hh = c // 2, c % 2
    hs = slice(hh * DH, (hh + 1) * DH)

    perm = np.empty(DH, np.int64)
    for lh in range(HPC):
        base = (hh * HPC + lh) * HD
        perm[lh * HD:lh * HD + HALF] = base + 2 * np.arange(HALF)
        perm[lh * HD + HALF:(lh + 1) * HD] = base + 2 * np.arange(HALF) + 1

    s = 0.125  # 1/sqrt(HD), folded into the q projection
    wq = np.ascontiguousarray((Wq[perm, :] * s).T).astype(BF16)
    wk = np.ascontiguousarray(Wk[perm, :].T).astype(BF16)
    wv = np.ascontiguousarray(Wv[hs, :].T).astype(BF16)

    p32 = np.arange(128) % 32
    cs2 = cos[:, p32].T.astype(BF16)
    sgn = np.where((np.arange(128) // 32) % 2 == 0, -1.0, 1.0).astype(np.float32)
    sn2 = (sin[:, p32] * sgn[None, :]).T.astype(BF16)

    kk = np.arange(128)[:, None]
    qq = np.arange(128)[None, :]
    tri = (kk <= qq)  # [128, 128] triangle for the partial diagonal block
    msk = np.repeat(tri[:, None, :], 2, axis=1).reshape(128, 256).astype(BF16)

    return {
        "xq": np.ascontiguousarray(q[b].T).astype(BF16),
        "xk": np.ascontiguousarray(k[b].T).astype(BF16),
        "xv": np.ascontiguousarray(v[b].T).astype(BF16),
        "wq": wq, "wk": wk, "wv": wv,
        "bqc": np.ascontiguousarray((bq[perm] * s).reshape(NT, 128).T, np.float32),
        "bkc": np.ascontiguousarray(bk[perm].reshape(NT, 128).T, np.float32),
        "cs2": cs2, "sn2": sn2, "msk": msk,
    }


def prep_in_maps(q, k, v, Wq, bq, Wk, bk, Wv, bv, sin, cos):
    args = [np.asarray(a, np.float32) for a in (q, k, v, Wq, bq, Wk, bk, Wv, bv, sin, cos)]
    maps = [_prep_core_inputs(c, *args) for c in range(8)]
    return maps, args[8]  # bv needed on host in assemble()


def assemble(results, bv):
    out = np.empty((BATCH, SEQ, EMB), np.float32)
    for c in range(8):
        b, hh = c // 2, c % 2
        outT = np.asarray(results[c]["outT"], np.float32)
        l = np.asarray(results[c]["lsum"], np.float32)
        a = outT.reshape(HPC, HD, SEQ) / l[:, None, :]
        out[b, :, hh * DH:(hh + 1) * DH] = a.reshape(DH, SEQ).T \
            + bv[hh * DH:(hh + 1) * DH][None, :]
    return out


def run(in_maps, causal=True, trace=False, **kw):
    _install_ntff_shim()
    from concourse.bass_utils import run_bass_kernel_spmd
    nc = build(causal)
    return run_bass_kernel_spmd(nc, in_maps, core_ids=list(range(8)), trace=trace, **kw)


def kernel(q, k, v, Wq, bq, Wk, bk, Wv, bv, sin, cos, mask):
    in_maps, bv_f = prep_in_maps(q, k, v, Wq, bq, Wk, bk, Wv, bv, sin, cos)
    r = run(in_maps, causal=bool(mask))
    return assemble(r.results, bv_f)


# revision 23
# speedup vs baseline: 1.3767x; 1.1362x over previous
"""Multi-head causal attention (QKV proj + RoPE + softmax) on 8 TRN2 NeuronCores.

Sharding: batch 4-way x head-group 2-way -> each core handles 1 batch and 8
contiguous heads (512 output channels). No collectives; host gathers slices.

Per-core algorithm (all matmul compute in bf16, fp32 PSUM accumulation):
  - host passes x.T (q/k/v of its batch, transposed to [emb, seq]) and W.T
    shards so every matmul contracts over the partition dim without on-device
    transposes.
  - q/k weights are row-permuted per head into [even dims | odd dims] so RoPE
    becomes: rot = x*cs + swap32(x)*sn, where swap32 is an SBUF partition-block
    swap done by DMA. The per-head dim permutation cancels in q.k dot products.
  - q/k biases are per-partition columns folded into the PSUM eviction
    (tensor_scalar add); the v bias is applied on host: P@(V+b) = P@V + l*b.
  - scores are computed transposed, S_T[k, q] = kh_T.T @ qh_T (K=64
    contraction; the A/B heads of a 128-row tile are emitted adjacently so
    they run concurrently on PE row groups).
  - softmax: exp on ScalarE from PSUM at [128,1024] granularity (no max
    subtraction: |scores| <= ~5 by construction), causal mask multiplies on
    DVE for the diagonal tiles only; fully-masked k-tiles are skipped.
  - attnT[d, q] = sum_kt V_tile[k,d|1].T @ P_T[k, q] -- a ones-column appended
    to V makes row 64 the softmax denominator for free.
  - unnormalized attnT and the denominator row go to HBM via one SBUF staging
    copy; division + final transpose + v-bias happen on host.

Scheduling (the perf rewrite vs the 313us baseline): instead of projections
-> attention phases, everything is one software pipeline. Attention units
start as soon as the m=0 projections land (~19us), and the remaining
projection pieces (8 matmul-pair quanta) are woven between the per-kt
score/exp/attnV steps so the PE never idles while ScalarE paces the exp
stream. DMA issue is split across the sync/gpsimd/vector queues so no
engine serializes input loading.
"""

import sys
import types
from collections import deque

import numpy as np
import ml_dtypes

BF16 = ml_dtypes.bfloat16
SEQ, EMB, NHEADS, BATCH = 2048, 1024, 16, 4
HD, HALF = 64, 32
HPC = 8          # heads per core
DH = 512         # output dims per core
NE = EMB // 128  # 8 contraction tiles
NT = 4           # head-pair (128-row) dout tiles
NKT = SEQ // 128  # 16 key tiles
NQC = SEQ // 512  # 4 query chunks


def _install_ntff_shim():
    """The image's antenv lacks axon_hooks; synthesize it from trn_agent_boot
    so run_bass_kernel_spmd(trace=True) can profile. Harmless if unused."""
    try:
        import antenv.axon_hooks  # noqa: F401
        return
    except ImportError:
        pass
    try:
        from trn_agent_boot.trn_boot import _ntff_profile_via_ctypes
        import antenv
    except ImportError:
        return
    hook = _ntff_profile_via_ctypes("/opt/axon/libaxon_pjrt.so")
    mod = types.ModuleType("antenv.axon_hooks")
    mod.get_axon_ntff_profile_hook = lambda: hook
    mod.set_axon_ntff_profile_hook = lambda h: None
    sys.modules["antenv.axon_hooks"] = mod
    antenv.axon_hooks = mod


_built = {}


def build(causal=True):
    if causal in _built:
        return _built[causal]
    import concourse.mybir as mybir
    import concourse.tile as tile
    from concourse import bacc

    f32 = mybir.dt.float32
    bf = mybir.dt.bfloat16
    EXP = mybir.ActivationFunctionType.Exp
    IDN = mybir.ActivationFunctionType.Identity
    CPY = mybir.ActivationFunctionType.Copy
    MUL = mybir.AluOpType.mult
    ADD = mybir.AluOpType.add

    nc = bacc.Bacc(None, target_bir_lowering=False, debug=False)
    with tile.TileContext(nc) as tc:
        with tc.tile_pool(name="dram", bufs=1, space="DRAM") as dram:
            xq_d = dram.tile([EMB, SEQ], bf, kind="ExternalInput", name="xq", uniquify=False)
            xk_d = dram.tile([EMB, SEQ], bf, kind="ExternalInput", name="xk", uniquify=False)
            xv_d = dram.tile([EMB, SEQ], bf, kind="ExternalInput", name="xv", uniquify=False)
            wq_d = dram.tile([EMB, DH], bf, kind="ExternalInput", name="wq", uniquify=False)
            wk_d = dram.tile([EMB, DH], bf, kind="ExternalInput", name="wk", uniquify=False)
            wv_d = dram.tile([EMB, DH], bf, kind="ExternalInput", name="wv", uniquify=False)
            bqc_d = dram.tile([128, NT], f32, kind="ExternalInput", name="bqc", uniquify=False)
            bkc_d = dram.tile([128, NT], f32, kind="ExternalInput", name="bkc", uniquify=False)
            cs_d = dram.tile([128, SEQ], bf, kind="ExternalInput", name="cs2", uniquify=False)
            sn_d = dram.tile([128, SEQ], bf, kind="ExternalInput", name="sn2", uniquify=False)
            mk_d = dram.tile([128, 256], bf, kind="ExternalInput", name="msk", uniquify=False)
            outT_d = dram.tile([DH, SEQ], f32, kind="ExternalOutput", name="outT", uniquify=False)
            l_d = dram.tile([HPC, SEQ], f32, kind="ExternalOutput", name="lsum", uniquify=False)

            with tc.tile_pool(name="const", bufs=1) as cp, \
                 tc.tile_pool(name="xq_p", bufs=16) as xqp, \
                 tc.tile_pool(name="xk_p", bufs=16) as xkp, \
                 tc.tile_pool(name="xv_p", bufs=16) as xvp, \
                 tc.tile_pool(name="rope", bufs=3) as rp, \
                 tc.tile_pool(name="ostage", bufs=3) as op, \
                 tc.tile_pool(name="pp", bufs=2, space="PSUM") as pp, \
                 tc.tile_pool(name="sp", bufs=2, space="PSUM") as sp, \
                 tc.tile_pool(name="tA", bufs=1, space="PSUM") as ptA, \
                 tc.tile_pool(name="tB", bufs=1, space="PSUM") as ptB:

                qh = cp.tile([128, NT, SEQ], bf, name="qh")
                kh = cp.tile([128, NT, SEQ], bf, name="kh")
                vsb = cp.tile([128, NKT, HPC * 65], bf, name="vsb")
                probs = cp.tile([128, 2, NKT, 512], bf, name="probs")
                w_sb = {n: cp.tile([128, NE, DH], bf, name=f"w_{n}") for n in "qkv"}
                b_sb = {n: cp.tile([128, NT], f32, name=f"b_{n}") for n in "qk"}
                cs = cp.tile([128, SEQ], bf, name="cs")
                sn = cp.tile([128, SEQ], bf, name="sn")
                msk = cp.tile([128, 2, 128], bf, name="mskt")

                # ---------------- DMA issue (queued upfront, 3 queues) ----
                # x inputs in [128, 512] sc-chunks so the first matmul's
                # operands land within ~7us and pool live-set stays small.
                xt = {"q": {}, "k": {}, "v": {}}
                pools = {"q": xqp, "k": xkp, "v": xvp}
                srcs = {"q": xq_d, "k": xk_d, "v": xv_d}

                def load_x(nm, e, c, eng):
                    t = pools[nm].tile([128, 512], bf, tag="x", name=f"x{nm}{e}_{c}")
                    eng.dma_start(out=t[:, :],
                                  in_=srcs[nm][e * 128:(e + 1) * 128,
                                               c * 512:(c + 1) * 512])
                    xt[nm][(e, c)] = t

                wd = {"q": wq_d, "k": wk_d, "v": wv_d}

                def load_w(nm, e, eng):
                    eng.dma_start(out=w_sb[nm][:, e, :],
                                  in_=wd[nm][e * 128:(e + 1) * 128, :])

                # sync queue: q/k weights+inputs, small constants; the
                # pool-recycled (hence dependency-gated) xq sc2/3 go last so
                # they cannot head-of-line-block anything. Output stores are
                # appended inline later.
                for e in range(NE):
                    load_w("q", e, nc.sync)
                    load_x("q", e, 0, nc.sync)
                nc.sync.dma_start(out=b_sb["q"][:, :], in_=bqc_d[:, :])
                nc.sync.dma_start(out=b_sb["k"][:, :], in_=bkc_d[:, :])
                for e in range(NE):
                    load_x("q", e, 1, nc.sync)
                nc.sync.dma_start(out=cs[:, :], in_=cs_d[:, :])
                nc.sync.dma_start(out=sn[:, :], in_=sn_d[:, :])
                nc.sync.dma_start(out=msk[:, :, :],
                                  in_=mk_d[:, :].rearrange("p (h u) -> p h u", h=2))
                for e in range(NE):
                    load_w("k", e, nc.sync)
                    load_x("k", e, 0, nc.sync)
                for e in range(NE):
                    load_x("k", e, 1, nc.sync)

                # gpsimd queue (slow SW-dynamic DMA, ~1.25us/descriptor):
                # v-side loads (not consumed until ~40us) followed by the
                # late pool-recycled loads, dependency-gated on first-wave
                # consumers anyway.
                for e in range(NE):
                    load_w("v", e, nc.gpsimd)
                    load_x("v", e, 0, nc.gpsimd)
                for e in range(NE):
                    load_x("v", e, 1, nc.gpsimd)
                for c in (2, 3):
                    for e in range(NE):
                        load_x("q", e, c, nc.gpsimd)
                for c in (2, 3):
                    for e in range(NE):
                        load_x("k", e, c, nc.gpsimd)
                    for e in range(NE):
                        load_x("v", e, c, nc.gpsimd)

                # only the ones-columns (col 64 of each 65-block) need init;
                # the v evictions overwrite the 64 data columns of every block
                nc.vector.memset(
                    vsb[:, :, :].rearrange("p k (h u) -> p k h u", u=65)[:, :, :, 64:65],
                    1.0)

                # ---------------- projection pieces ----------------------
                # tmp tiles per (nm, m, half): filled by two sc pieces, then
                # roped into qh/kh and released.
                tmps = {}

                def proj_piece(nm, m, sc):
                    # one [128out, 512seq] quantum: 8 e-tile matmul pairs
                    h, c = sc // 2, sc % 2
                    key = (nm, m, h)
                    if key not in tmps:
                        tmps[key] = rp.tile([128, 1024], bf, tag="tmp", bufs=10,
                                            name=f"tp{nm}{m}{h}")
                    tmp = tmps[key]
                    ps = pp.tile([128, 512], f32, tag="p", name=f"pp{nm}{m}{sc}")
                    for e in range(NE):
                        nc.tensor.matmul(
                            ps[0:64, :],
                            w_sb[nm][:, e, m * 128:m * 128 + 64],
                            xt[nm][(e, sc)][:, :],
                            start=(e == 0), stop=(e == NE - 1))
                        nc.tensor.matmul(
                            ps[64:128, :],
                            w_sb[nm][:, e, m * 128 + 64:(m + 1) * 128],
                            xt[nm][(e, sc)][:, :],
                            start=(e == 0), stop=(e == NE - 1))
                    # PSUM eviction + bias on the Scalar engine (idle during
                    # the projection phase; DVE keeps only the rope muls)
                    nc.scalar.activation(
                        tmp[:, c * 512:(c + 1) * 512], ps[:, :], IDN,
                        bias=b_sb[nm][:, m:m + 1])

                def rope_half(nm, m, h):
                    # consumes tmp(nm, m, h) -> writes dst[:, m, h*1024:+1024]
                    dst = qh if nm == "q" else kh
                    tmp = tmps.pop((nm, m, h))
                    lo, hi = h * 1024, (h + 1) * 1024
                    tsw = rp.tile([128, 1024], bf, tag="tsw", name=f"tw{nm}{m}{h}")
                    for blk in range(4):
                        s = blk ^ 1
                        nc.scalar.dma_start(out=tsw[blk * 32:(blk + 1) * 32, :],
                                            in_=tmp[s * 32:(s + 1) * 32, :])
                    m2 = rp.tile([128, 1024], bf, tag="m2", name=f"m2{nm}{m}{h}")
                    nc.vector.tensor_tensor(dst[:, m, lo:hi], tmp[:, :], cs[:, lo:hi], MUL)
                    nc.vector.tensor_tensor(m2[:, :], tsw[:, :], sn[:, lo:hi], MUL)
                    nc.vector.tensor_tensor(dst[:, m, lo:hi], dst[:, m, lo:hi], m2[:, :], ADD)

                def v_piece(sp2):
                    # two st (128-seq) tiles of the v projection
                    for u in range(2):
                        st = 2 * sp2 + u
                        o = st * 128
                        sc, col = o // 512, o % 512
                        ps = pp.tile([128, 512], f32, tag="p", name=f"ppv{st}")
                        for e in range(NE):
                            nc.tensor.matmul(
                                ps[0:64, :],
                                xt["v"][(e, sc)][:, col:col + 64],
                                w_sb["v"][:, e, :],
                                start=(e == 0), stop=(e == NE - 1))
                            nc.tensor.matmul(
                                ps[64:128, :],
                                xt["v"][(e, sc)][:, col + 64:col + 128],
                                w_sb["v"][:, e, :],
                                start=(e == 0), stop=(e == NE - 1))
                        nc.scalar.activation(
                            vsb[:, st, :]
                            .rearrange("p (h u) -> p h u", u=65)[:, :, 0:64],
                            ps[:, :].rearrange("p (h d) -> p h d", d=64), CPY)

                # ---------------- filler plumbing ------------------------
                def emit(tok):
                    kind = tok[0]
                    if kind == "Q":
                        proj_piece("q", tok[1], tok[2])
                    elif kind == "K":
                        proj_piece("k", tok[1], tok[2])
                    elif kind == "V":
                        v_piece(tok[1])
                    elif kind == "RQ":
                        rope_half("q", tok[1], tok[2])
                    elif kind == "RK":
                        rope_half("k", tok[1], tok[2])
                    done.add(tok)

                done = set()

                # ---------------- attention unit steps -------------------
                pt = {}

                def unit_kt(t, j, kt, nkt):
                    if kt == 0:
                        pt[0] = ptA.tile([65, 512], f32, tag="t0", name=f"pt0_{t}{j}")
                        pt[1] = ptB.tile([65, 512], f32, tag="t1", name=f"pt1_{t}{j}")
                    # causal column shrink: for diagonal tiles, query columns
                    # below o are fully masked -- skip them in the scores
                    # matmul (moving N), the exp, and the attnV accumulation
                    # (nested ranges, so PSUM accumulate stays consistent).
                    dd = kt - 4 * j if causal else -1
                    o = 128 * max(dd, 0)
                    ps = sp.tile([128, 1024], f32, tag="s", name=f"ps{t}{j}_{kt}")
                    # A/B heads write the tile's two different PSUM banks
                    # from PE row groups 0/1 -> they run concurrently.
                    for half in (0, 1):
                        po = half * 64
                        nc.tensor.matmul(
                            ps[:, half * 512 + o:(half + 1) * 512],
                            kh[po:po + 64, t, kt * 128:(kt + 1) * 128],
                            qh[po:po + 64, t, j * 512 + o:(j + 1) * 512],
                            start=True, stop=True)
                    nc.scalar.activation(
                        probs[:, :, kt, o:],
                        ps[:, :].rearrange("p (h u) -> p h u", h=2)[:, :, o:], EXP)
                    if causal and 0 <= dd:
                        # triangle mask on the single partial 128-col block
                        nc.vector.tensor_tensor(
                            probs[:, :, kt, o:o + 128], probs[:, :, kt, o:o + 128],
                            msk[:, :, :], MUL)
                    for half in (0, 1):
                        lh = 2 * t + half
                        nc.tensor.matmul(
                            pt[half][:, o:],
                            vsb[:, kt, lh * 65:(lh + 1) * 65],
                            probs[:, half, kt, o:],
                            start=(kt == 0), stop=(kt == nkt - 1))

                def unit_end(t, j):
                    for half in (0, 1):
                        lh = 2 * t + half
                        ost = op.tile([65, 512], f32, tag="ost", name=f"os{half}_{t}{j}")
                        nc.vector.tensor_copy(ost[:, :], pt[half][:, :])
                        nc.sync.dma_start(
                            out=outT_d[lh * 64:(lh + 1) * 64, j * 512:(j + 1) * 512],
                            in_=ost[0:64, :])
                        nc.sync.dma_start(
                            out=l_d[lh:lh + 1, j * 512:(j + 1) * 512],
                            in_=ost[64:65, :])

                # ---------------- the schedule ---------------------------
                # Phase 1: all projections as one contiguous hot matmul
                # stream (the PE only reaches full clock in long
                # uninterrupted bursts). Rope/evictions ride on DVE behind.
                # sc-wave-major order: all consumers of the sc0/1 x-chunks
                # run before any sc2/3 piece, so the x pools recycle without
                # blocking the in-order PE queue.
                for m in range(NT):
                    emit(("Q", m, 0)); emit(("Q", m, 1)); emit(("RQ", m, 0))
                for m in range(NT):
                    emit(("K", m, 0)); emit(("K", m, 1)); emit(("RK", m, 0))
                for sp2 in range(4):
                    emit(("V", sp2))
                for m in range(NT):
                    emit(("Q", m, 2)); emit(("Q", m, 3)); emit(("RQ", m, 1))
                for m in range(NT):
                    emit(("K", m, 2)); emit(("K", m, 3)); emit(("RK", m, 1))
                for sp2 in range(4, NKT // 2):
                    emit(("V", sp2))

                # Phase 2: pure exp-paced attention stream.
                order = [(t, j) for j in range(NQC) for t in range(NT)]
                for (t, j) in order:
                    nkt = 4 * (j + 1) if causal else NKT
                    for kt in range(nkt):
                        unit_kt(t, j, kt, nkt)
                    unit_end(t, j)
    _built[causal] = nc
    nc.compile()
    return nc


def _prep_core_inputs(c, q, k, v, Wq, bq, Wk, bk, Wv, bv, sin, cos):
    b, hh = c // 2, c % 2
    hs = slice(hh * DH, (hh + 1) * DH)

    perm = np.empty(DH, np.int64)
    for lh in range(HPC):
        base = (hh * HPC + lh) * HD
        perm[lh * HD:lh * HD + HALF] = base + 2 * np.arange(HALF)
        perm[lh * HD + HALF:(lh + 1) * HD] = base + 2 * np.arange(HALF) + 1

    s = 0.125  # 1/sqrt(HD), folded into the q projection
    wq = np.ascontiguousarray((Wq[perm, :] * s).T).astype(BF16)
    wk = np.ascontiguousarray(Wk[perm, :].T).astype(BF16)
    wv = np.ascontiguousarray(Wv[hs, :].T).astype(BF16)

    p32 = np.arange(128) % 32
    cs2 = cos[:, p32].T.astype(BF16)
    sgn = np.where((np.arange(128) // 32) % 2 == 0, -1.0, 1.0).astype(np.float32)
    sn2 = (sin[:, p32] * sgn[None, :]).T.astype(BF16)

    kk = np.arange(128)[:, None]
    qq = np.arange(128)[None, :]
    tri = (kk <= qq)  # [128, 128] triangle for the partial diagonal block
    msk = np.repeat(tri[:, None, :], 2, axis=1).reshape(128, 256).astype(BF16)

    return {
        "xq": np.ascontiguousarray(q[b].T).astype(BF16),
        "xk": np.ascontiguousarray(k[b].T).astype(BF16),
        "xv": np.ascontiguousarray(v[b].T).astype(BF16),
        "wq": wq, "wk": wk, "wv": wv,
        "bqc": np.ascontiguousarray((bq[perm] * s).reshape(NT, 128).T, np.float32),
        "bkc": np.ascontiguousarray(bk[perm].reshape(NT, 128).T, np.float32),
        "cs2": cs2, "sn2": sn2, "msk": msk,
    }


def prep_in_maps(q, k, v, Wq, bq, Wk, bk, Wv, bv, sin, cos):
    args = [np.asarray(a, np.float32) for a in (q, k, v, Wq, bq, Wk, bk, Wv, bv, sin, cos)]
    maps = [_prep_core_inputs(c, *args) for c in range(8)]
    return maps, args[8]  # bv needed on host in assemble()


def assemble(results, bv):
    out = np.empty((BATCH, SEQ, EMB), np.float32)
    for c in range(8):
        b, hh = c // 2, c % 2
        outT = np.asarray(results[c]["outT"], np.float32)
        l = np.asarray(results[c]["lsum"], np.float32)
        a = outT.reshape(HPC, HD, SEQ) / l[:, None, :]
        out[b, :, hh * DH:(hh + 1) * DH] = a.reshape(DH, SEQ).T \
            + bv[hh * DH:(hh + 1) * DH][None, :]
    return out


def run(in_maps, causal=True, trace=False, **kw):
    _install_ntff_shim()
    from concourse.bass_utils import run_bass_kernel_spmd
    nc = build(causal)
    return run_bass_kernel_spmd(nc, in_maps, core_ids=list(range(8)), trace=trace, **kw)


def kernel(q, k, v, Wq, bq, Wk, bk, Wv, bv, sin, cos, mask):
    in_maps, bv_f = prep_in_maps(q, k, v, Wq, bq, Wk, bk, Wv, bv, sin, cos)
    r = run(in_maps, causal=bool(mask))
    return assemble(r.results, bv_f)
